# revision 8
# baseline (speedup 1.0000x reference)
"""ConvergedInhibition forward on 8 Trainium2 NeuronCores.

The reference computes, independently for every (n, h, w) pixel, a
frequency-domain deconvolution along the channel axis C=128:

    out = ifft(fft(x, axis=C) / Fk).real

Division by Fk in frequency space is circular convolution with
g = ifft(1/Fk) (real, since delta-k is real), i.e. a fixed 128x128
circulant matrix M applied to every channel vector:

    out[n, :, h, w] = M @ x[n, :, h, w],   M[c, c'] = g[(c - c') mod C]

So the heavy work is a tiny stationary matmul swept over a 134 MB
activation tensor -> memory-bound tensor-engine kernel. The length-128
filter preprocessing (FFT of a 128-vector) is negligible and done on
host in float64.

Sharding: data-parallel over batch N=64 -> 8 batches per core, no
cross-core communication. The 2e-2 rel-err gate admits bfloat16 I/O
(~4e-3 measured), which halves HBM traffic vs fp32 — the binding
constraint: the fp32 version measured at the ~358 GB/s/core HBM
roofline (93.5 us), and per-core streaming tops out ~390 GB/s with all
8 cores active.

Per-core schedule: the host hands each core its slice pre-transposed
to a flat (C, 32768) panel so DMA granularity is free. Input streams
in 8 ascending-width pieces (small first so the first matmul starts
~4 us earlier), all resident in SBUF (no ring-reuse edges). One
standalone LDWEIGHTS loads the stationary inverse-circulant into the
PE; the 64 512-col matmuls skip the per-instruction weight reload
(562 -> ~450 ns per chunk) so the PE tracks the in-stream instead of
lagging it. PSUM drains split DVE/ACT per out-block with the ACT
chunks last, so each out-DMA follows its drains in program order on
the scalar HWDGE queue; out-blocks taper at the end to keep the final
drain-out chain off the critical path.
"""

import ml_dtypes
import numpy as np

import concourse.bass as bass
import concourse.mybir as mybir
from concourse import bacc
from concourse.bass_utils import run_bass_kernel_spmd
from concourse.tile import TileContext

N_CORES = 8
PSUM_CHUNK = 512  # fp32 elements per PSUM bank


def _inverse_circulant_lhsT(filt: np.ndarray, C: int) -> np.ndarray:
    """Build the stationary matmul operand lhsT (K x M layout).

    out[m] = sum_k M[m, k] x[k] with M[m, k] = g[(m - k) mod C], and the
    tensor engine computes lhsT.T @ rhs, so lhsT[k, m] = g[(m - k) mod C].
    """
    scope = filt.shape[-1]
    pad_left = (C - scope) // 2
    k = np.zeros(C, dtype=np.float64)
    k[pad_left : pad_left + scope] = filt.reshape(-1).astype(np.float64)
    k = np.roll(k, C // 2 + 1)
    delta = np.zeros(C, dtype=np.float64)
    delta[0] = 1.0
    g = np.fft.ifft(1.0 / np.fft.fft(delta - k)).real
    j = np.arange(C)
    return g[(j[None, :] - j[:, None]) % C].astype(np.float32)


def build_nc(C: int, M: int, io: str = "bf16") -> bacc.Bacc:
    mm_dt = {
        "bf16": mybir.dt.bfloat16,
        "f32": mybir.dt.float32,
        "f32r": mybir.dt.float32r,
    }[io]
    out_dt = mybir.dt.bfloat16 if io == "bf16" else mybir.dt.float32
    nc = bacc.Bacc("TRN2", target_bir_lowering=False, debug=False)
    x = nc.dram_tensor("x", [C, M], mm_dt, kind="ExternalInput")
    w = nc.dram_tensor("w", [C, C], mm_dt, kind="ExternalInput")
    y = nc.dram_tensor("y", [C, M], out_dt, kind="ExternalOutput")

    cw = PSUM_CHUNK
    # Ascending-width input pieces: the first matmul waits only on 128 KB.
    in_widths = [cw, cw, 2 * cw, 4 * cw, 8 * cw, 16 * cw, 16 * cw, 16 * cw]
    assert sum(in_widths) == M
    # Descending-width output blocks: the last drain->out chain is short.
    out_widths = [8 * cw] * 7 + [4 * cw, 2 * cw, cw, cw]
    assert sum(out_widths) == M

    with TileContext(nc) as tc:
        with (
            tc.tile_pool(name="wp", bufs=1) as wp,
            tc.tile_pool(name="xp", bufs=1) as xp,
            tc.tile_pool(name="yp", bufs=1) as yp,
            tc.tile_pool(name="pp", bufs=8, space="PSUM") as pp,
        ):
            wt = wp.tile([C, C], mm_dt)
            nc.sync.dma_start(wt[:], w[:, :])
            pieces = []
            off = 0
            for i, pw in enumerate(in_widths):
                t = xp.tile([C, pw], mm_dt, tag=f"x{i}", bufs=1)
                nc.sync.dma_start(t[:], x[:, bass.ds(off, pw)])
                pieces.append((t, off, pw))
                off += pw

            if io == "bf16":
                nc.tensor.ldweights(wt[:])
            yoff = 0
            for i, ow in enumerate(out_widths):
                yt = yp.tile([C, ow], out_dt, tag=f"y{i}", bufs=1)
                n_ch = ow // cw
                for j in range(n_ch):
                    col0 = yoff + j * cw
                    xt, poff, pw = next(
                        p for p in pieces if p[1] <= col0 < p[1] + p[2]
                    )
                    pt = pp.tile([C, cw], mybir.dt.float32)
                    mm = nc.tensor.matmul(
                        pt[:],
                        wt[:],
                        xt[:, bass.ds(col0 - poff, cw)],
                        start=True,
                        stop=True,
                    )
                    if io == "bf16":
                        # Stationary weights never change: one LDWEIGHTS
                        # above, every matmul skips the reload. (fp32/f32r
                        # can't: walrus miscompiles non-self-loading 4-byte
                        # matmuls.)
                        mm.ins.ldweights = False
                    # PSUM has no DMA route: drain via both copy engines,
                    # interleaved even/odd so each block drains at 2x and
                    # neither engine serializes a whole block.
                    cols = bass.ds(j * cw, cw)
                    if j % 2 == 0:
                        nc.vector.tensor_copy(yt[:, cols], pt[:])
                    else:
                        nc.scalar.copy(yt[:, cols], pt[:])
                # Out-DMAs ride the sync HWDGE queue, which is idle once the
                # in-DMAs are issued. Issuing them from the ACT queue was
                # measured to convoy: descriptor-gen blocked ACT's drains,
                # which stalled the PE through the PSUM ring.
                nc.sync.dma_start(y[:, bass.ds(yoff, ow)], yt[:])
                yoff += ow
    nc.compile()
    return nc


_NC_CACHE: dict = {}


def _run(activations, inhibition_filter, use_f32r=False, io=None, **spmd_kwargs):
    act = np.ascontiguousarray(np.asarray(activations, dtype=np.float32))
    filt = np.asarray(inhibition_filter, dtype=np.float32)
    B, C, H, W = act.shape
    P = H * W
    assert B % N_CORES == 0
    b_per_core = B // N_CORES
    M = b_per_core * P
    if io is None:
        io = "f32r" if use_f32r else "bf16"

    lhsT = _inverse_circulant_lhsT(filt, C)
    key = (C, M, io)
    nc = _NC_CACHE.get(key)
    if nc is None:
        nc = _NC_CACHE[key] = build_nc(C, M, io=io)

    in_dt = ml_dtypes.bfloat16 if io == "bf16" else np.float32
    # (N_CORES, b, C, P) -> per-core flat (C, b*P) panels
    xs = act.reshape(N_CORES, b_per_core, C, P).transpose(0, 2, 1, 3)
    xs = np.ascontiguousarray(xs.reshape(N_CORES, C, M), dtype=in_dt)
    lhsT = lhsT.astype(in_dt)
    in_maps = [{"x": xs[i], "w": lhsT} for i in range(N_CORES)]
    res = run_bass_kernel_spmd(nc, in_maps, core_ids=list(range(N_CORES)), **spmd_kwargs)
    out = np.stack([res.results[i]["y"] for i in range(N_CORES)], axis=0)
    out = out.reshape(N_CORES, C, b_per_core, P).transpose(0, 2, 1, 3)
    return np.ascontiguousarray(out.reshape(B, C, H, W), dtype=np.float32), res


def kernel(activations: np.ndarray, inhibition_filter: np.ndarray) -> np.ndarray:
    out, _ = _run(activations, inhibition_filter)
    return out


# revision 9
# speedup vs baseline: 1.0068x; 1.0068x over previous
"""ConvergedInhibition forward on 8 Trainium2 NeuronCores.

The reference computes, independently for every (n, h, w) pixel, a
frequency-domain deconvolution along the channel axis C=128:

    out = ifft(fft(x, axis=C) / Fk).real

Division by Fk in frequency space is circular convolution with
g = ifft(1/Fk) (real, since delta-k is real), i.e. a fixed 128x128
circulant matrix M applied to every channel vector:

    out[n, :, h, w] = M @ x[n, :, h, w],   M[c, c'] = g[(c - c') mod C]

So the heavy work is a tiny stationary matmul swept over a 134 MB
activation tensor -> memory-bound tensor-engine kernel. The length-128
filter preprocessing (FFT of a 128-vector) is negligible and done on
host in float64.

Sharding: data-parallel over batch N=64 -> 8 batches per core, no
cross-core communication. The 2e-2 rel-err gate admits bfloat16 I/O
(~4e-3 measured), which halves HBM traffic vs fp32 — the binding
constraint: the fp32 version measured at the ~358 GB/s/core HBM
roofline (93.5 us), and per-core streaming tops out ~390 GB/s with all
8 cores active.

Per-core schedule: the host hands each core its slice pre-transposed
to a flat (C, 32768) panel so DMA granularity is free. Input streams
in 8 ascending-width pieces (small first so the first matmul starts
~4 us earlier), all resident in SBUF (no ring-reuse edges). One
standalone LDWEIGHTS loads the stationary inverse-circulant into the
PE; the 64 512-col matmuls skip the per-instruction weight reload
(562 -> ~450 ns per chunk) so the PE tracks the in-stream instead of
lagging it. PSUM drains split DVE/ACT per out-block with the ACT
chunks last, so each out-DMA follows its drains in program order on
the scalar HWDGE queue; out-blocks taper at the end to keep the final
drain-out chain off the critical path.
"""

import ml_dtypes
import numpy as np

import concourse.bass as bass
import concourse.mybir as mybir
from concourse import bacc
from concourse.bass_utils import run_bass_kernel_spmd
from concourse.tile import TileContext

N_CORES = 8
PSUM_CHUNK = 512  # fp32 elements per PSUM bank


def _inverse_circulant_lhsT(filt: np.ndarray, C: int) -> np.ndarray:
    """Build the stationary matmul operand lhsT (K x M layout).

    out[m] = sum_k M[m, k] x[k] with M[m, k] = g[(m - k) mod C], and the
    tensor engine computes lhsT.T @ rhs, so lhsT[k, m] = g[(m - k) mod C].
    """
    scope = filt.shape[-1]
    pad_left = (C - scope) // 2
    k = np.zeros(C, dtype=np.float64)
    k[pad_left : pad_left + scope] = filt.reshape(-1).astype(np.float64)
    k = np.roll(k, C // 2 + 1)
    delta = np.zeros(C, dtype=np.float64)
    delta[0] = 1.0
    g = np.fft.ifft(1.0 / np.fft.fft(delta - k)).real
    j = np.arange(C)
    return g[(j[None, :] - j[:, None]) % C].astype(np.float32)


def build_nc(C: int, M: int, io: str = "bf16") -> bacc.Bacc:
    mm_dt = {
        "bf16": mybir.dt.bfloat16,
        "f32": mybir.dt.float32,
        "f32r": mybir.dt.float32r,
    }[io]
    out_dt = mybir.dt.bfloat16 if io == "bf16" else mybir.dt.float32
    nc = bacc.Bacc("TRN2", target_bir_lowering=False, debug=False)
    x = nc.dram_tensor("x", [C, M], mm_dt, kind="ExternalInput")
    w = nc.dram_tensor("w", [C, C], mm_dt, kind="ExternalInput")
    y = nc.dram_tensor("y", [C, M], out_dt, kind="ExternalOutput")

    cw = PSUM_CHUNK
    # Ascending-width input pieces: the first matmul waits only on 128 KB.
    in_widths = [cw, cw, 2 * cw, 4 * cw, 8 * cw, 16 * cw, 16 * cw, 16 * cw]
    assert sum(in_widths) == M
    # Uniform 1 MB output blocks: sub-512 KB DMAs spread over only 1-2 of
    # the 16 DMA engines (~25 GB/s each), so a "short" tapered tail was
    # measured 10 us SLOWER than one more full-width block.
    out_widths = [8 * cw] * (M // (8 * cw))
    assert sum(out_widths) == M

    with TileContext(nc) as tc:
        with (
            tc.tile_pool(name="wp", bufs=1) as wp,
            tc.tile_pool(name="xp", bufs=1) as xp,
            tc.tile_pool(name="yp", bufs=1) as yp,
            tc.tile_pool(name="pp", bufs=8, space="PSUM") as pp,
        ):
            wt = wp.tile([C, C], mm_dt)
            nc.sync.dma_start(wt[:], w[:, :])
            pieces = []
            off = 0
            for i, pw in enumerate(in_widths):
                t = xp.tile([C, pw], mm_dt, tag=f"x{i}", bufs=1)
                nc.sync.dma_start(t[:], x[:, bass.ds(off, pw)])
                pieces.append((t, off, pw))
                off += pw

            if io == "bf16":
                nc.tensor.ldweights(wt[:])
            yoff = 0
            for i, ow in enumerate(out_widths):
                yt = yp.tile([C, ow], out_dt, tag=f"y{i}", bufs=1)
                n_ch = ow // cw
                for j in range(n_ch):
                    col0 = yoff + j * cw
                    xt, poff, pw = next(
                        p for p in pieces if p[1] <= col0 < p[1] + p[2]
                    )
                    pt = pp.tile([C, cw], mybir.dt.float32)
                    mm = nc.tensor.matmul(
                        pt[:],
                        wt[:],
                        xt[:, bass.ds(col0 - poff, cw)],
                        start=True,
                        stop=True,
                    )
                    if io == "bf16":
                        # Stationary weights never change: one LDWEIGHTS
                        # above, every matmul skips the reload. (fp32/f32r
                        # can't: walrus miscompiles non-self-loading 4-byte
                        # matmuls.)
                        mm.ins.ldweights = False
                    # PSUM has no DMA route: drain via both copy engines,
                    # interleaved even/odd so each block drains at 2x and
                    # neither engine serializes a whole block.
                    cols = bass.ds(j * cw, cw)
                    if j % 2 == 0:
                        nc.vector.tensor_copy(yt[:, cols], pt[:])
                    else:
                        nc.scalar.copy(yt[:, cols], pt[:])
                # Out-DMAs ride the sync HWDGE queue, which is idle once the
                # in-DMAs are issued. Issuing them from the ACT queue was
                # measured to convoy: descriptor-gen blocked ACT's drains,
                # which stalled the PE through the PSUM ring.
                nc.sync.dma_start(y[:, bass.ds(yoff, ow)], yt[:])
                yoff += ow
    nc.compile()
    return nc


_NC_CACHE: dict = {}


def _run(activations, inhibition_filter, use_f32r=False, io=None, **spmd_kwargs):
    act = np.ascontiguousarray(np.asarray(activations, dtype=np.float32))
    filt = np.asarray(inhibition_filter, dtype=np.float32)
    B, C, H, W = act.shape
    P = H * W
    assert B % N_CORES == 0
    b_per_core = B // N_CORES
    M = b_per_core * P
    if io is None:
        io = "f32r" if use_f32r else "bf16"

    lhsT = _inverse_circulant_lhsT(filt, C)
    key = (C, M, io)
    nc = _NC_CACHE.get(key)
    if nc is None:
        nc = _NC_CACHE[key] = build_nc(C, M, io=io)

    in_dt = ml_dtypes.bfloat16 if io == "bf16" else np.float32
    # (N_CORES, b, C, P) -> per-core flat (C, b*P) panels
    xs = act.reshape(N_CORES, b_per_core, C, P).transpose(0, 2, 1, 3)
    xs = np.ascontiguousarray(xs.reshape(N_CORES, C, M), dtype=in_dt)
    lhsT = lhsT.astype(in_dt)
    in_maps = [{"x": xs[i], "w": lhsT} for i in range(N_CORES)]
    res = run_bass_kernel_spmd(nc, in_maps, core_ids=list(range(N_CORES)), **spmd_kwargs)
    out = np.stack([res.results[i]["y"] for i in range(N_CORES)], axis=0)
    out = out.reshape(N_CORES, C, b_per_core, P).transpose(0, 2, 1, 3)
    return np.ascontiguousarray(out.reshape(B, C, H, W), dtype=np.float32), res


def kernel(activations: np.ndarray, inhibition_filter: np.ndarray) -> np.ndarray:
    out, _ = _run(activations, inhibition_filter)
    return out


# revision 10
# speedup vs baseline: 1.0307x; 1.0237x over previous
"""ConvergedInhibition forward on 8 Trainium2 NeuronCores.

The reference computes, independently for every (n, h, w) pixel, a
frequency-domain deconvolution along the channel axis C=128:

    out = ifft(fft(x, axis=C) / Fk).real

Division by Fk in frequency space is circular convolution with
g = ifft(1/Fk) (real, since delta-k is real), i.e. a fixed 128x128
circulant matrix M applied to every channel vector:

    out[n, :, h, w] = M @ x[n, :, h, w],   M[c, c'] = g[(c - c') mod C]

So the heavy work is a tiny stationary matmul swept over a 134 MB
activation tensor -> memory-bound tensor-engine kernel. The length-128
filter preprocessing (FFT of a 128-vector) is negligible and done on
host in float64.

Sharding: data-parallel over batch N=64 -> 8 batches per core, no
cross-core communication. The 2e-2 rel-err gate admits bfloat16 I/O
(~4e-3 measured), which halves HBM traffic vs fp32 — the binding
constraint: the fp32 version measured at the ~358 GB/s/core HBM
roofline (93.5 us), and per-core streaming tops out ~390 GB/s with all
8 cores active.

Per-core schedule: the host hands each core its slice pre-transposed
to a flat (C, 32768) panel so DMA granularity is free. Input streams
in 8 ascending-width pieces (small first so the first matmul starts
~4 us earlier), all resident in SBUF (no ring-reuse edges). One
standalone LDWEIGHTS loads the stationary inverse-circulant into the
PE; the 64 512-col matmuls skip the per-instruction weight reload
(562 -> ~450 ns per chunk) so the PE tracks the in-stream instead of
lagging it. PSUM drains split DVE/ACT per out-block with the ACT
chunks last, so each out-DMA follows its drains in program order on
the scalar HWDGE queue; out-blocks taper at the end to keep the final
drain-out chain off the critical path.
"""

import ml_dtypes
import numpy as np

import concourse.bass as bass
import concourse.mybir as mybir
from concourse import bacc
from concourse.bass_utils import run_bass_kernel_spmd
from concourse.tile import TileContext

N_CORES = 8
PSUM_CHUNK = 512  # fp32 elements per PSUM bank


def _inverse_circulant_lhsT(filt: np.ndarray, C: int) -> np.ndarray:
    """Build the stationary matmul operand lhsT (K x M layout).

    out[m] = sum_k M[m, k] x[k] with M[m, k] = g[(m - k) mod C], and the
    tensor engine computes lhsT.T @ rhs, so lhsT[k, m] = g[(m - k) mod C].
    """
    scope = filt.shape[-1]
    pad_left = (C - scope) // 2
    k = np.zeros(C, dtype=np.float64)
    k[pad_left : pad_left + scope] = filt.reshape(-1).astype(np.float64)
    k = np.roll(k, C // 2 + 1)
    delta = np.zeros(C, dtype=np.float64)
    delta[0] = 1.0
    g = np.fft.ifft(1.0 / np.fft.fft(delta - k)).real
    j = np.arange(C)
    return g[(j[None, :] - j[:, None]) % C].astype(np.float32)


def build_nc(C: int, M: int, io: str = "bf16") -> bacc.Bacc:
    mm_dt = {
        "bf16": mybir.dt.bfloat16,
        "f32": mybir.dt.float32,
        "f32r": mybir.dt.float32r,
    }[io]
    out_dt = mybir.dt.bfloat16 if io == "bf16" else mybir.dt.float32
    nc = bacc.Bacc("TRN2", target_bir_lowering=False, debug=False)
    x = nc.dram_tensor("x", [C, M], mm_dt, kind="ExternalInput")
    w = nc.dram_tensor("w", [C, C], mm_dt, kind="ExternalInput")
    y = nc.dram_tensor("y", [C, M], out_dt, kind="ExternalOutput")

    cw = PSUM_CHUNK
    # Ascending-width input pieces: the first matmul waits only on 128 KB.
    in_widths = [cw, cw, 2 * cw, 4 * cw, 8 * cw, 16 * cw, 16 * cw, 16 * cw]
    assert sum(in_widths) == M
    # Uniform 1 MB output blocks: sub-512 KB DMAs spread over only 1-2 of
    # the 16 DMA engines (~25 GB/s each), so a "short" tapered tail was
    # measured 10 us SLOWER than one more full-width block.
    out_widths = [8 * cw] * (M // (8 * cw))
    assert sum(out_widths) == M

    with TileContext(nc) as tc:
        with (
            tc.tile_pool(name="wp", bufs=1) as wp,
            tc.tile_pool(name="xp", bufs=1) as xp,
            tc.tile_pool(name="yp", bufs=1) as yp,
            tc.tile_pool(name="pp", bufs=8, space="PSUM") as pp,
        ):
            wt = wp.tile([C, C], mm_dt)
            nc.sync.dma_start(wt[:], w[:, :])
            pieces = []
            off = 0
            for i, pw in enumerate(in_widths):
                t = xp.tile([C, pw], mm_dt, tag=f"x{i}", bufs=1)
                nc.sync.dma_start(t[:], x[:, bass.ds(off, pw)])
                pieces.append((t, off, pw))
                off += pw

            if io == "bf16":
                nc.tensor.ldweights(wt[:])
            yoff = 0
            for i, ow in enumerate(out_widths):
                yt = yp.tile([C, ow], out_dt, tag=f"y{i}", bufs=1)
                n_ch = ow // cw
                for j in range(n_ch):
                    col0 = yoff + j * cw
                    xt, poff, pw = next(
                        p for p in pieces if p[1] <= col0 < p[1] + p[2]
                    )
                    pt = pp.tile([C, cw], mybir.dt.float32)
                    mm = nc.tensor.matmul(
                        pt[:],
                        wt[:],
                        xt[:, bass.ds(col0 - poff, cw)],
                        start=True,
                        stop=True,
                    )
                    if io == "bf16":
                        # Stationary weights never change: one LDWEIGHTS
                        # above, every matmul skips the reload. (fp32/f32r
                        # can't: walrus miscompiles non-self-loading 4-byte
                        # matmuls.)
                        mm.ins.ldweights = False
                    # PSUM has no DMA route: drain via both copy engines,
                    # interleaved (not half/half: serializing a block's
                    # drains on one engine was the old 3.7us/block convoy).
                    # DVE takes 5 chunks, ACT 3 + the out-DMA descriptor
                    # gen, and ACT gets the block-final chunk so the out
                    # issue follows its own queue's drains in program order.
                    cols = bass.ds(j * cw, cw)
                    if j in (3, 5, 7) and n_ch == 8:
                        nc.scalar.copy(yt[:, cols], pt[:])
                    else:
                        nc.vector.tensor_copy(yt[:, cols], pt[:])
                # Out-DMAs MUST ride the scalar engine's HWDGE queue: each
                # DMA engine has separate import (Q_I) and export (Q_X)
                # rings, and only scalar-queue DMAs use the export rings.
                # Routing outs via the sync queue stacked 16.8 MB onto the
                # import rings and serialized the tail (~+3 us measured).
                nc.scalar.dma_start(y[:, bass.ds(yoff, ow)], yt[:])
                yoff += ow
    nc.compile()
    return nc


_NC_CACHE: dict = {}


def _run(activations, inhibition_filter, use_f32r=False, io=None, **spmd_kwargs):
    act = np.ascontiguousarray(np.asarray(activations, dtype=np.float32))
    filt = np.asarray(inhibition_filter, dtype=np.float32)
    B, C, H, W = act.shape
    P = H * W
    assert B % N_CORES == 0
    b_per_core = B // N_CORES
    M = b_per_core * P
    if io is None:
        io = "f32r" if use_f32r else "bf16"

    lhsT = _inverse_circulant_lhsT(filt, C)
    key = (C, M, io)
    nc = _NC_CACHE.get(key)
    if nc is None:
        nc = _NC_CACHE[key] = build_nc(C, M, io=io)

    in_dt = ml_dtypes.bfloat16 if io == "bf16" else np.float32
    # (N_CORES, b, C, P) -> per-core flat (C, b*P) panels
    xs = act.reshape(N_CORES, b_per_core, C, P).transpose(0, 2, 1, 3)
    xs = np.ascontiguousarray(xs.reshape(N_CORES, C, M), dtype=in_dt)
    lhsT = lhsT.astype(in_dt)
    in_maps = [{"x": xs[i], "w": lhsT} for i in range(N_CORES)]
    res = run_bass_kernel_spmd(nc, in_maps, core_ids=list(range(N_CORES)), **spmd_kwargs)
    out = np.stack([res.results[i]["y"] for i in range(N_CORES)], axis=0)
    out = out.reshape(N_CORES, C, b_per_core, P).transpose(0, 2, 1, 3)
    return np.ascontiguousarray(out.reshape(B, C, H, W), dtype=np.float32), res


def kernel(activations: np.ndarray, inhibition_filter: np.ndarray) -> np.ndarray:
    out, _ = _run(activations, inhibition_filter)
    return out


# revision 14
# speedup vs baseline: 1.4940x; 1.4496x over previous
"""ConvergedInhibition forward on 8 Trainium2 NeuronCores.

The reference computes, independently for every (n, h, w) pixel, a
frequency-domain deconvolution along the channel axis C=128:

    out = ifft(fft(x, axis=C) / Fk).real

Division by Fk in frequency space is circular convolution with
g = ifft(1/Fk) (real, since delta-k is real), i.e. a fixed 128x128
circulant matrix M applied to every channel vector:

    out[n, :, h, w] = M @ x[n, :, h, w],   M[c, c'] = g[(c - c') mod C]

So the heavy work is a tiny stationary matmul swept over a 134 MB
activation tensor -> memory-bound tensor-engine kernel. The length-128
filter preprocessing (FFT of a 128-vector) is negligible and done on
host in float64.

Sharding: data-parallel over batch N=64 -> 8 batches per core, no
cross-core communication. The 2e-2 rel-err gate admits bfloat16 I/O
(~4e-3 measured), which halves HBM traffic vs fp32 — the binding
constraint: the fp32 version measured at the ~358 GB/s/core HBM
roofline (93.5 us), and per-core streaming tops out ~390 GB/s with all
8 cores active.

Per-core schedule: the host hands each core its slice pre-transposed
to a flat (C, 32768) panel so DMA granularity is free. Input streams
in 8 ascending-width pieces (small first so the first matmul starts
~4 us earlier), all resident in SBUF (no ring-reuse edges). One
standalone LDWEIGHTS loads the stationary inverse-circulant into the
PE; the 64 512-col matmuls skip the per-instruction weight reload
(562 -> ~450 ns per chunk) so the PE tracks the in-stream instead of
lagging it. PSUM drains split DVE/ACT per out-block with the ACT
chunks last, so each out-DMA follows its drains in program order on
the scalar HWDGE queue; out-blocks taper at the end to keep the final
drain-out chain off the critical path.
"""

import ml_dtypes
import numpy as np

import concourse.bass as bass
import concourse.mybir as mybir
from concourse import bacc
from concourse.bass_utils import run_bass_kernel_spmd
from concourse.tile import TileContext

N_CORES = 8
PSUM_CHUNK = 512  # fp32 elements per PSUM bank


def _inverse_circulant_lhsT(filt: np.ndarray, C: int) -> np.ndarray:
    """Build the stationary matmul operand lhsT (K x M layout).

    out[m] = sum_k M[m, k] x[k] with M[m, k] = g[(m - k) mod C], and the
    tensor engine computes lhsT.T @ rhs, so lhsT[k, m] = g[(m - k) mod C].
    """
    scope = filt.shape[-1]
    pad_left = (C - scope) // 2
    k = np.zeros(C, dtype=np.float64)
    k[pad_left : pad_left + scope] = filt.reshape(-1).astype(np.float64)
    k = np.roll(k, C // 2 + 1)
    delta = np.zeros(C, dtype=np.float64)
    delta[0] = 1.0
    g = np.fft.ifft(1.0 / np.fft.fft(delta - k)).real
    j = np.arange(C)
    return g[(j[None, :] - j[:, None]) % C].astype(np.float32)


def build_nc(C: int, M: int, io: str = "fp8") -> bacc.Bacc:
    # io="fp8": residual form. The device streams x as fp8e4 and returns
    # only the correction c = (M - I) @ x as fp8e4 — 8.4 MB/core instead
    # of fp32's 33.6 — and the host adds back the exact x it already
    # holds. Quantization error only touches c (||c||/||y|| = 0.16), so
    # the measured rel err is ~6e-3 against the 2e-2 gate.
    in_dt = {
        "fp8": mybir.dt.float8e4,
        "bf16": mybir.dt.bfloat16,
        "f32": mybir.dt.float32,
        "f32r": mybir.dt.float32r,
    }[io]
    w_dt = {
        "fp8": mybir.dt.bfloat16,  # tiny stationary operand: keep precision
        "bf16": mybir.dt.bfloat16,
        "f32": mybir.dt.float32,
        "f32r": mybir.dt.float32r,
    }[io]
    out_dt = {
        "fp8": mybir.dt.float8e4,
        "bf16": mybir.dt.bfloat16,
        "f32": mybir.dt.float32,
        "f32r": mybir.dt.float32,
    }[io]
    nc = bacc.Bacc("TRN2", target_bir_lowering=False, debug=False)
    x = nc.dram_tensor("x", [C, M], in_dt, kind="ExternalInput")
    w = nc.dram_tensor("w", [C, C], w_dt, kind="ExternalInput")
    y = nc.dram_tensor("y", [C, M], out_dt, kind="ExternalOutput")

    cw = PSUM_CHUNK
    # Ascending-width input pieces: the first matmul waits only on 128 KB.
    in_widths = [cw, cw, 2 * cw, 4 * cw, 8 * cw, 16 * cw, 16 * cw, 16 * cw]
    assert sum(in_widths) == M
    # Uniform 1 MB output blocks: sub-512 KB DMAs spread over only 1-2 of
    # the 16 DMA engines (~25 GB/s each), so a "short" tapered tail was
    # measured 10 us SLOWER than one more full-width block.
    out_widths = [8 * cw] * (M // (8 * cw))
    assert sum(out_widths) == M

    with TileContext(nc) as tc:
        with (
            tc.tile_pool(name="wp", bufs=1) as wp,
            tc.tile_pool(name="xp", bufs=1) as xp,
            tc.tile_pool(name="yp", bufs=1) as yp,
            tc.tile_pool(name="pp", bufs=8, space="PSUM") as pp,
        ):
            wt = wp.tile([C, C], w_dt)
            nc.sync.dma_start(wt[:], w[:, :])
            pieces = []
            off = 0
            for i, pw in enumerate(in_widths):
                t = xp.tile([C, pw], in_dt, tag=f"x{i}", bufs=1)
                nc.sync.dma_start(t[:], x[:, bass.ds(off, pw)])
                pieces.append((t, off, pw))
                off += pw

            elide_ldw = io in ("bf16", "fp8")
            if elide_ldw:
                nc.tensor.ldweights(wt[:])
            yoff = 0
            for i, ow in enumerate(out_widths):
                yt = yp.tile([C, ow], out_dt, tag=f"y{i}", bufs=1)
                n_ch = ow // cw
                for j in range(n_ch):
                    col0 = yoff + j * cw
                    xt, poff, pw = next(
                        p for p in pieces if p[1] <= col0 < p[1] + p[2]
                    )
                    pt = pp.tile([C, cw], mybir.dt.float32)
                    mm = nc.tensor.matmul(
                        pt[:],
                        wt[:],
                        xt[:, bass.ds(col0 - poff, cw)],
                        start=True,
                        stop=True,
                    )
                    if elide_ldw:
                        # Stationary weights never change: one LDWEIGHTS
                        # above, every matmul skips the reload. (fp32/f32r
                        # can't: walrus miscompiles non-self-loading 4-byte
                        # matmuls.)
                        mm.ins.ldweights = False
                    # PSUM has no DMA route: drain via both copy engines,
                    # interleaved (not half/half: serializing a block's
                    # drains on one engine was the old 3.7us/block convoy).
                    # DVE takes 5 chunks, ACT 3 + the out-DMA descriptor
                    # gen, and ACT gets the block-final chunk so the out
                    # issue follows its own queue's drains in program order.
                    cols = bass.ds(j * cw, cw)
                    if j in (3, 5, 7) and n_ch == 8:
                        nc.scalar.copy(yt[:, cols], pt[:])
                    else:
                        nc.vector.tensor_copy(yt[:, cols], pt[:])
                # Out-DMAs MUST ride the scalar engine's HWDGE queue: each
                # DMA engine has separate import (Q_I) and export (Q_X)
                # rings, and only scalar-queue DMAs use the export rings.
                # Routing outs via the sync queue stacked 16.8 MB onto the
                # import rings and serialized the tail (~+3 us measured).
                nc.scalar.dma_start(y[:, bass.ds(yoff, ow)], yt[:])
                yoff += ow
    nc.compile()
    return nc


_NC_CACHE: dict = {}


def _run(activations, inhibition_filter, use_f32r=False, io=None, **spmd_kwargs):
    act = np.ascontiguousarray(np.asarray(activations, dtype=np.float32))
    filt = np.asarray(inhibition_filter, dtype=np.float32)
    B, C, H, W = act.shape
    P = H * W
    assert B % N_CORES == 0
    b_per_core = B // N_CORES
    M = b_per_core * P
    if io is None:
        io = "f32r" if use_f32r else "fp8"

    lhsT = _inverse_circulant_lhsT(filt, C)
    key = (C, M, io)
    nc = _NC_CACHE.get(key)
    if nc is None:
        nc = _NC_CACHE[key] = build_nc(C, M, io=io)

    residual = io == "fp8"
    if residual:
        in_dt, w_dt = ml_dtypes.float8_e4m3fn, ml_dtypes.bfloat16
        lhsT = lhsT - np.eye(C, dtype=np.float32)  # device computes c = (M-I)x
    elif io == "bf16":
        in_dt = w_dt = ml_dtypes.bfloat16
    else:
        in_dt = w_dt = np.float32
    # (N_CORES, b, C, P) -> per-core flat (C, b*P) panels
    xs = act.reshape(N_CORES, b_per_core, C, P).transpose(0, 2, 1, 3)
    xs = np.ascontiguousarray(xs.reshape(N_CORES, C, M), dtype=in_dt)
    in_maps = [{"x": xs[i], "w": lhsT.astype(w_dt)} for i in range(N_CORES)]
    res = run_bass_kernel_spmd(nc, in_maps, core_ids=list(range(N_CORES)), **spmd_kwargs)
    out = np.stack([res.results[i]["y"] for i in range(N_CORES)], axis=0)
    out = out.reshape(N_CORES, C, b_per_core, P).transpose(0, 2, 1, 3)
    out = np.ascontiguousarray(out.reshape(B, C, H, W), dtype=np.float32)
    if residual:
        out += act
    return out, res


def kernel(activations: np.ndarray, inhibition_filter: np.ndarray) -> np.ndarray:
    out, _ = _run(activations, inhibition_filter)
    return out
